# revision 3
# baseline (speedup 1.0000x reference)
"""Trainium2 Bass kernel for the CAB (context-attention-block) fusion:

    out = shallower * sigmoid(MLP(concat(gap(shallower), gap(deeper)))) +
          bilinear_upsample_2x(deeper)

Sharding: pure data parallel — batch 16 is split 2-per-core across 8
NeuronCores; the tiny 1x1-conv weights are replicated.  Each core:

  - loads its deeper [2,256,32,32] and shallower [2,256,64,64] slices into
    SBUF with channels on partitions (2 channel groups of 128),
  - global-average-pools both via ScalarE (ACT) accumulate,
  - runs the 2-layer MLP on the TensorE (weights pre-transposed and
    pre-scaled on host so raw sums feed the matmul directly),
  - bilinearly upsamples deeper 32x32 -> 64x64 with two separable passes,
    computed as y = 3*a + b forms on VectorE plus a *1/16 scaled copy on
    ScalarE (the 2x bilinear kernel weights are {0.75, 0.25} = {3,1}/4),
  - fuses the sigmoid gating and residual add in one scalar_tensor_tensor,
  - stores the result.

Numerics: fp32 end to end.
"""

import numpy as np
from contextlib import ExitStack

import concourse.bacc as bacc
import concourse.tile as tile
import concourse.mybir as mybir
from concourse import bass_utils

F32 = mybir.dt.float32
AF = mybir.ActivationFunctionType
OP = mybir.AluOpType

N_CORES = 8
B, C = 16, 256
HD, WD = 32, 32
HS, WS = 64, 64
BL = B // N_CORES          # batches per core
G = C // 128               # channel groups of 128


def _emit(ctx, tc, deeper, shallower, w1t, w2t, b1, b2, out):
    nc = tc.nc

    wpool = ctx.enter_context(tc.tile_pool(name="weights", bufs=1))
    stat = ctx.enter_context(tc.tile_pool(name="stat", bufs=1))
    sres = ctx.enter_context(tc.tile_pool(name="sres", bufs=1))
    dres = ctx.enter_context(tc.tile_pool(name="dres", bufs=1))
    up = ctx.enter_context(tc.tile_pool(name="up", bufs=2))
    psum = ctx.enter_context(tc.tile_pool(name="psum", bufs=1, space="PSUM"))

    # ---- weights / biases (replicated, tiny) ----
    w1sb = []
    for k in range(4):
        w1c = wpool.tile([128, C], F32, name=f"w1sb{k}")
        nc.sync.dma_start(w1c[:], w1t[k * 128:(k + 1) * 128, :])
        w1sb.append(w1c)
    w2sb = []
    for k in range(2):
        w2c = wpool.tile([128, C], F32, name=f"w2sb{k}")
        nc.sync.dma_start(w2c[:], w2t[k * 128:(k + 1) * 128, :])
        w2sb.append(w2c)
    b1sb, b2sb = [], []
    for g in range(G):
        bb1 = wpool.tile([128, 1], F32, name=f"b1sb{g}")
        nc.sync.dma_start(bb1[:], b1[g * 128:(g + 1) * 128, :])
        b1sb.append(bb1)
        bb2 = wpool.tile([128, 1], F32, name=f"b2sb{g}")
        nc.sync.dma_start(bb2[:], b2[g * 128:(g + 1) * 128, :])
        b2sb.append(bb2)

    # ---- deeper loads first: the upsample (bulk of DVE work) only needs these
    d_sb = {}
    for b in range(BL):
        for g in range(G):
            dt_ = dres.tile([128, HD, WD], F32, name=f"d{b}{g}")
            nc.sync.dma_start(dt_[:], deeper[b, g * 128:(g + 1) * 128, :, :])
            d_sb[b, g] = dt_

    s_sb = {}
    for b in range(BL):
        for g in range(G):
            st = sres.tile([128, HS, WS], F32, name=f"s{b}{g}")
            gs = slice(g * 128, (g + 1) * 128)
            half = HS // 2
            nc.sync.dma_start(st[:, 0:half, :], shallower[b, gs, 0:half, :])
            nc.sync.dma_start(st[:, half:HS, :], shallower[b, gs, half:HS, :])
            s_sb[b, g] = st

    # ---- global average pools: ACT in-place copy with free-axis accumulate.
    # Raw sums only; the 1/(H*W) scales are folded into w1t on the host.
    spool = [stat.tile([128, BL], F32, name=f"spool{g}") for g in range(G)]
    dpool = [stat.tile([128, BL], F32, name=f"dpool{g}") for g in range(G)]
    for b in range(BL):
        for g in range(G):
            nc.scalar.activation(d_sb[b, g][:], d_sb[b, g][:], AF.Copy,
                                 accum_out=dpool[g][:, b:b + 1])
            nc.scalar.activation(s_sb[b, g][:], s_sb[b, g][:], AF.Copy,
                                 accum_out=spool[g][:, b:b + 1])

    # ---- MLP: gate = sigmoid(w2 @ relu(w1 @ gp + b1) + b2), channels on
    # partitions so the gate lands as per-partition scalars.
    gp_chunks = [spool[0], spool[1], dpool[0], dpool[1]]
    h_sb = []
    for og in range(G):
        ph = psum.tile([128, BL], F32, name=f"ph{og}")
        for kc in range(4):
            nc.tensor.matmul(ph[:], w1sb[kc][:, og * 128:(og + 1) * 128],
                             gp_chunks[kc][:], start=(kc == 0), stop=(kc == 3))
        ht = stat.tile([128, BL], F32, name=f"h{og}")
        nc.scalar.activation(ht[:], ph[:], AF.Relu, bias=b1sb[og][:])
        h_sb.append(ht)
    sig = []
    for g in range(G):
        pg = psum.tile([128, BL], F32, name=f"pg{g}")
        for ig in range(G):
            nc.tensor.matmul(pg[:], w2sb[ig][:, g * 128:(g + 1) * 128],
                             h_sb[ig][:], start=(ig == 0), stop=(ig == 1))
        sg = stat.tile([128, BL], F32, name=f"sig{g}")
        nc.scalar.activation(sg[:], pg[:], AF.Sigmoid, bias=b2sb[g][:])
        sig.append(sg)

    # ---- per-tile: 2x bilinear upsample of deeper, then fused gate+add.
    # W pass builds yp = 4 * w_upsample(x); H pass builds u = upsampled/1.
    for b in range(BL):
        for g in range(G):
            d = d_sb[b, g]
            yp = up.tile([128, HD, WS], F32, name="yp")
            ypv = yp.rearrange("p h (j t) -> p h j t", t=2)
            # W pass: even w=2j (j>=1): 3x[j]+x[j-1]; odd w=2j+1 (j<=30):
            # 3x[j]+x[j+1]; edges copy*4.
            nc.vector.scalar_tensor_tensor(
                ypv[:, :, 1:WD, 0], d[:, :, 1:WD], 3.0, d[:, :, 0:WD - 1],
                OP.mult, OP.add)
            nc.vector.scalar_tensor_tensor(
                ypv[:, :, 0:WD - 1, 1], d[:, :, 0:WD - 1], 3.0, d[:, :, 1:WD],
                OP.mult, OP.add)
            nc.scalar.mul(ypv[:, :, 0, 0], d[:, :, 0], 4.0)
            nc.scalar.mul(ypv[:, :, WD - 1, 1], d[:, :, WD - 1], 4.0)

            # H pass on yp rows; ACT applies the deferred 1/16.
            te = up.tile([128, HD - 1, WS], F32, name="te")
            to = up.tile([128, HD - 1, WS], F32, name="to")
            nc.vector.scalar_tensor_tensor(
                te[:], yp[:, 1:HD, :], 3.0, yp[:, 0:HD - 1, :], OP.mult, OP.add)
            nc.vector.scalar_tensor_tensor(
                to[:], yp[:, 0:HD - 1, :], 3.0, yp[:, 1:HD, :], OP.mult, OP.add)
            u = up.tile([128, HS, WS], F32, name="u")
            uv = u.rearrange("p (i t) w -> p i t w", t=2)
            nc.scalar.mul(uv[:, 1:HD, 0, :], te[:], 1.0 / 16.0)
            nc.scalar.mul(uv[:, 0:HD - 1, 1, :], to[:], 1.0 / 16.0)
            nc.scalar.mul(uv[:, 0, 0, :], yp[:, 0, :], 0.25)
            nc.scalar.mul(uv[:, HD - 1, 1, :], yp[:, HD - 1, :], 0.25)

            # out = shallower * sigmoid(gate) + upsample, in place in s tile.
            s = s_sb[b, g]
            nc.vector.scalar_tensor_tensor(
                s[:], s[:], sig[g][:, b:b + 1], u[:], OP.mult, OP.add)

            gs = slice(g * 128, (g + 1) * 128)
            half = HS // 2
            nc.sync.dma_start(out[b, gs, 0:half, :], s[:, 0:half, :])
            nc.sync.dma_start(out[b, gs, half:HS, :], s[:, half:HS, :])


def build_kernel():
    nc = bacc.Bacc("TRN2", target_bir_lowering=False, debug=False,
                   num_devices=N_CORES)
    deeper = nc.dram_tensor("deeper", [BL, C, HD, WD], F32,
                            kind="ExternalInput").ap()
    shallower = nc.dram_tensor("shallower", [BL, C, HS, WS], F32,
                               kind="ExternalInput").ap()
    w1t = nc.dram_tensor("w1t", [2 * C, C], F32, kind="ExternalInput").ap()
    w2t = nc.dram_tensor("w2t", [C, C], F32, kind="ExternalInput").ap()
    b1 = nc.dram_tensor("b1", [C, 1], F32, kind="ExternalInput").ap()
    b2 = nc.dram_tensor("b2", [C, 1], F32, kind="ExternalInput").ap()
    out = nc.dram_tensor("out", [BL, C, HS, WS], F32,
                         kind="ExternalOutput").ap()

    with tile.TileContext(nc) as tc, ExitStack() as ctx:
        _emit(ctx, tc, deeper, shallower, w1t, w2t, b1, b2, out)
    nc.compile()
    return nc


_NC = None


def _get_nc():
    global _NC
    if _NC is None:
        _NC = build_kernel()
    return _NC


def prepare_in_maps(deeper, shallower, w1, b1, w2, b2):
    w1t = np.ascontiguousarray(w1.T).astype(np.float32)
    w1t[:C] *= np.float32(1.0 / (HS * WS))   # shallow-pool rows: mean fold
    w1t[C:] *= np.float32(1.0 / (HD * WD))   # deeper-pool rows: mean fold
    w2t = np.ascontiguousarray(w2.T).astype(np.float32)
    b1c = np.ascontiguousarray(b1.reshape(C, 1)).astype(np.float32)
    b2c = np.ascontiguousarray(b2.reshape(C, 1)).astype(np.float32)
    in_maps = []
    for i in range(N_CORES):
        in_maps.append({
            "deeper": np.ascontiguousarray(deeper[i * BL:(i + 1) * BL]),
            "shallower": np.ascontiguousarray(shallower[i * BL:(i + 1) * BL]),
            "w1t": w1t, "w2t": w2t, "b1": b1c, "b2": b2c,
        })
    return in_maps


def gather(results):
    return np.concatenate([results[i]["out"] for i in range(N_CORES)], axis=0)


def kernel(deeper, shallower, w1, b1, w2, b2):
    nc = _get_nc()
    in_maps = prepare_in_maps(deeper, shallower, w1, b1, w2, b2)
    res = bass_utils.run_bass_kernel_spmd(nc, in_maps, list(range(N_CORES)))
    return gather(res.results)
